# revision 1
# baseline (speedup 1.0000x reference)
"""Trainium2 Bass kernel for AvgReadout-style segment mean + L2 normalize.

reference:
    vsum[i] = sum over edges e with src[e]==i of emb[dst[e]]
    deg[i]  = count of such edges (clamped to >=1)
    out     = l2_normalize(vsum / deg, eps=1e-12)

Key identity: l2_normalize(vsum/deg) == l2_normalize(vsum) whenever deg >= 1
(positive per-row scalar doesn't change direction), and for deg == 0 both are
exactly 0.  So the kernel only needs vsum, never deg.

Distribution: edges are sorted by src on host and sharded by src-range across
8 cores (12500 segments each).  Each core's output slice is disjoint, so no
collectives are needed.

Per core the segments are processed in 98 blocks of 128.  Edge rows are
fetched with dma_gather (int16 indices, so emb is addressed as 4 quarter
tables of 25000 rows); edges are grouped into cells (block, quarter), padded
to whole subtiles of 128 edges.  Cell capacities are maxed across cores so a
single compiled program serves all 8 cores.  Blocks are processed in
superblocks of 4; within a superblock the subtiles are quarter-major so each
dma_gather call (<=1024 indices, the SWDGE ring limit; round-robined over 4
SWDGE queues to parallelize Q7 descriptor generation) reads one quarter
table.  Per subtile a one-hot (edge -> segment-in-block) matrix built on DVE
feeds a PE matmul accumulating into the block's PSUM tile [128 segs, 128
feat].  Pad edges carry an out-of-range srcloc sentinel so their one-hot
column is all zeros.  Epilogue per block: sum-of-squares (ACT Square+accum),
sqrt, clamp 1e-12, reciprocal, scale-copy, DMA out.
"""

import numpy as np
from contextlib import ExitStack

N_SPOT = 100000
D = 128
P = 128
NCORES = 8
SEG_PER_CORE = 12500
NBLK = (SEG_PER_CORE + P - 1) // P  # 98
NQ = 4            # emb quarter tables (int16 index range)
QROWS = N_SPOT // NQ  # 25000
SB = 4            # blocks per superblock (PSUM tiles live concurrently)
CALL_CAP = 8      # subtiles per dma_gather call (1024 idx = SWDGE ring limit)
NQUEUES = 4       # SWDGE queues to round-robin
PAD_SENTINEL = 999.0


def compute_layout(capsub):
    """capsub: [nblk, NQ] int array of per-cell subtile capacities.

    Returns dict with:
      nslots: total subtile slots
      slot_block: per-slot block id
      calls: list of (q, slot0, nsub) dma_gather calls, in slot order
      blk_slots: per-block list of slot ids (ascending)
      sb_list: list of (blocks, calls_idx) per superblock
    """
    capsub = np.asarray(capsub)
    nblk = capsub.shape[0]
    slot_block = []
    calls = []
    blk_slots = [[] for _ in range(nblk)]
    sb_list = []
    slot = 0
    for sb0 in range(0, nblk, SB):
        blocks = list(range(sb0, min(sb0 + SB, nblk)))
        call_lo = len(calls)
        for q in range(NQ):
            run = []  # slots of this (superblock, q) run
            for b in blocks:
                for _ in range(int(capsub[b, q])):
                    slot_block.append(b)
                    blk_slots[b].append(slot)
                    run.append(slot)
                    slot += 1
            for i in range(0, len(run), CALL_CAP):
                chunk = run[i : i + CALL_CAP]
                calls.append((q, chunk[0], len(chunk)))
        sb_list.append((blocks, (call_lo, len(calls))))
    return {
        "nslots": slot,
        "slot_block": slot_block,
        "calls": calls,
        "blk_slots": blk_slots,
        "sb_list": sb_list,
    }


def preprocess(emb, mask, ncores=NCORES, seg_per_core=SEG_PER_CORE, nblk=NBLK,
               nspot=N_SPOT):
    """Sort/shard/pad edges. Returns (in_maps, capsub, layout)."""
    qrows = nspot // NQ
    emb = np.ascontiguousarray(np.asarray(emb, dtype=np.float32))
    emb16 = emb.astype(np.float16)
    mask = np.asarray(mask)
    src = mask[0].astype(np.int64, copy=False)
    dst = mask[1].astype(np.int64, copy=False)

    order = np.argsort(src, kind="stable")
    src_s = src[order].astype(np.int32)
    dst_s = dst[order].astype(np.int32)

    core_bounds = np.searchsorted(
        src_s, (seg_per_core * np.arange(ncores + 1)).astype(np.int32)
    )

    percore = []
    cnts = np.zeros((ncores, nblk * NQ), np.int64)
    for k in range(ncores):
        lo, hi = int(core_bounds[k]), int(core_bounds[k + 1])
        s = src_s[lo:hi] - seg_per_core * k
        d = dst_s[lo:hi]
        cell = (s >> 7) * NQ + d // qrows
        o = np.lexsort((d, cell))
        s, d, cell = s[o], d[o], cell[o]
        cnts[k] = np.bincount(cell, minlength=nblk * NQ)
        percore.append((s, d, cell))

    capsub = (-(-cnts.max(axis=0) // P)).reshape(nblk, NQ).astype(np.int64)
    layout = compute_layout(capsub)
    nslots = layout["nslots"]

    # slot base per cell, following the layout's slot order
    cell_slot0 = np.zeros(nblk * NQ, np.int64)
    nxt = np.zeros(nblk * NQ, np.int64)
    slot_of_cell = {}
    # reconstruct per-cell slot bases: slots are assigned per (sb, q, b) in
    # capsub order; walk the same order.
    slot = 0
    for sb0 in range(0, nblk, SB):
        for q in range(NQ):
            for b in range(sb0, min(sb0 + SB, nblk)):
                cell_slot0[b * NQ + q] = slot
                slot += int(capsub[b, q])
    assert slot == nslots

    iota = np.broadcast_to(np.arange(P, dtype=np.float16)[None, :], (P, P)).copy()

    in_maps = []
    for k in range(ncores):
        s, d, cell = percore[k]
        cum = np.zeros(nblk * NQ, np.int64)
        cc = cnts[k]
        cum[1:] = np.cumsum(cc)[:-1]
        rank = np.arange(len(s), dtype=np.int64) - cum[cell]
        pos = cell_slot0[cell] * P + rank  # global edge position

        srcloc = np.full(nslots * P, PAD_SENTINEL, np.float16)
        srcloc[pos] = (s & 127).astype(np.float16)
        dloc = np.zeros(nslots * P, np.int16)
        dloc[pos] = (d % qrows).astype(np.int16)

        # srcloc tile [p, slot] = value of edge (slot, p)
        srcloc_t = np.ascontiguousarray(srcloc.reshape(nslots, P).T)
        # idx16 [j%16, slot*8 + j//16] = dloc of edge (slot, j), replicated
        # across the 8 partition groups for the Q7 ucode.
        idx_blk = np.ascontiguousarray(
            dloc.reshape(nslots * 8, 16).T
        )  # [16, nslots*8]
        idx16 = np.tile(idx_blk, (8, 1))
        in_maps.append(
            {"emb": emb16, "srcloc": srcloc_t, "dstidx": idx16, "iota": iota}
        )
    return in_maps, capsub, layout


def build_program(capsub, layout, nblk=NBLK, nspot=N_SPOT, d=D, repeats=1):
    import concourse.bass as bass
    import concourse.tile as tile
    from concourse import bacc, mybir

    qrows = nspot // NQ
    nslots = layout["nslots"]
    calls = layout["calls"]
    blk_slots = layout["blk_slots"]
    sb_list = layout["sb_list"]

    nc = bacc.Bacc(
        "TRN2", target_bir_lowering=False, debug=False, num_swdge_queues=NQUEUES
    )
    emb_t = nc.dram_tensor("emb", [nspot, d], mybir.dt.float16, kind="ExternalInput")
    srcloc_t = nc.dram_tensor(
        "srcloc", [P, nslots], mybir.dt.float16, kind="ExternalInput"
    )
    dstidx_t = nc.dram_tensor(
        "dstidx", [P, nslots * 8], mybir.dt.int16, kind="ExternalInput"
    )
    iota_t = nc.dram_tensor("iota", [P, P], mybir.dt.float16, kind="ExternalInput")
    out_t = nc.dram_tensor(
        "out", [nblk * P, d], mybir.dt.float32, kind="ExternalOutput"
    )

    # slot -> (call idx, position within call)
    slot_call = [None] * nslots
    for ci, (q, s0, nsub) in enumerate(calls):
        for t in range(nsub):
            slot_call[s0 + t] = (ci, t)

    with tile.TileContext(nc) as tc, ExitStack() as ctx:
        consts = ctx.enter_context(tc.tile_pool(name="consts", bufs=1))
        gpool = ctx.enter_context(tc.tile_pool(name="gather", bufs=24))
        ohpool = ctx.enter_context(tc.tile_pool(name="onehot", bufs=24))
        spool = ctx.enter_context(tc.tile_pool(name="scratch", bufs=4))
        opool = ctx.enter_context(tc.tile_pool(name="outs", bufs=4))
        ppool = ctx.enter_context(tc.tile_pool(name="psum", bufs=8, space="PSUM"))

        srcloc_sb = consts.tile([P, nslots], mybir.dt.float16)
        nc.sync.dma_start(srcloc_sb[:], srcloc_t.ap())
        dstidx_sb = consts.tile([P, nslots * 8], mybir.dt.int16)
        nc.sync.dma_start(dstidx_sb[:], dstidx_t.ap())
        iota_sb = consts.tile([P, P], mybir.dt.float16)
        nc.sync.dma_start(iota_sb[:], iota_t.ap())

        out_ap = out_t.ap()
        emb_ap = emb_t.ap()
        callno = 0
        for rep in range(repeats):
            for blocks, (clo, chi) in sb_list:
                gtiles = {}
                ohtiles = {}
                for ci in range(clo, chi):
                    q, s0, nsub = calls[ci]
                    gt = gpool.tile([P, CALL_CAP * d], mybir.dt.float16, tag="gt")
                    nc.gpsimd.dma_gather(
                        out_ap=gt[:, : nsub * d].rearrange(
                            "p (c e) -> p c e", e=d
                        ),
                        in_ap=emb_ap[q * qrows : (q + 1) * qrows, :],
                        idxs_ap=dstidx_sb[:, s0 * 8 : (s0 + nsub) * 8],
                        num_idxs=nsub * P,
                        num_idxs_reg=nsub * P,
                        elem_size=d,
                        single_packet=False,
                        queue_num=callno % NQUEUES,
                    )
                    gtiles[ci] = gt
                    callno += 1
                    # one batched one-hot build for the call's subtiles:
                    # oh[p, t, c] = (iota[p, c] == srcloc[p, s0+t])
                    oh = ohpool.tile([P, CALL_CAP * P], mybir.dt.float16, tag="oh")
                    oh3 = oh[:, : nsub * P].rearrange("p (t c) -> p t c", c=P)
                    iota_b = bass.AP(
                        iota_sb[:].tensor,
                        iota_sb[:].offset,
                        [iota_sb[:].ap[0], [0, nsub], [1, P]],
                    )
                    srl = srcloc_sb[:, s0 : s0 + nsub]
                    srl_b = bass.AP(
                        srl.tensor, srl.offset, [srl.ap[0], [1, nsub], [0, P]]
                    )
                    nc.vector.tensor_tensor(
                        out=oh3,
                        in0=iota_b,
                        in1=srl_b,
                        op=mybir.AluOpType.is_equal,
                    )
                    ohtiles[ci] = oh
                for b in blocks:
                    slots = blk_slots[b]
                    if not slots:
                        ot = opool.tile([P, d], mybir.dt.float32)
                        nc.vector.memset(ot[:], 0.0)
                        nc.sync.dma_start(out_ap[b * P : (b + 1) * P, :], ot[:])
                        continue
                    ps = ppool.tile([P, d], mybir.dt.float32, space="PSUM")
                    for i, sl in enumerate(slots):
                        ci, t = slot_call[sl]
                        nc.tensor.matmul(
                            ps[:],
                            lhsT=ohtiles[ci][:, t * P : (t + 1) * P],
                            rhs=gtiles[ci][:, t * d : (t + 1) * d],
                            start=(i == 0),
                            stop=(i == len(slots) - 1),
                        )
                    sq = spool.tile([P, d], mybir.dt.float32)
                    ss = spool.tile([P, 1], mybir.dt.float32)
                    nc.scalar.activation(
                        sq[:],
                        ps[:],
                        mybir.ActivationFunctionType.Square,
                        accum_out=ss[:],
                    )
                    nrm = spool.tile([P, 1], mybir.dt.float32)
                    nc.scalar.activation(
                        nrm[:], ss[:], mybir.ActivationFunctionType.Sqrt
                    )
                    nc.vector.tensor_scalar(
                        out=nrm[:],
                        in0=nrm[:],
                        scalar1=1e-12,
                        scalar2=None,
                        op0=mybir.AluOpType.max,
                    )
                    nc.vector.reciprocal(nrm[:], nrm[:])
                    ot = opool.tile([P, d], mybir.dt.float32)
                    nc.scalar.activation(
                        ot[:],
                        ps[:],
                        mybir.ActivationFunctionType.Copy,
                        scale=nrm[:],
                    )
                    nc.sync.dma_start(out_ap[b * P : (b + 1) * P, :], ot[:])

    nc.compile()
    return nc


_PROGRAM_CACHE = {}


def _get_program(capsub, layout):
    key = capsub.tobytes()
    if key not in _PROGRAM_CACHE:
        _PROGRAM_CACHE[key] = build_program(capsub, layout)
    return _PROGRAM_CACHE[key]


def kernel(**inputs):
    emb = inputs["emb"]
    mask = inputs["mask"]
    in_maps, capsub, layout = preprocess(emb, mask)
    nc = _get_program(capsub, layout)

    from concourse.bass_utils import run_bass_kernel_spmd

    res = run_bass_kernel_spmd(nc, in_maps, core_ids=list(range(NCORES)))
    out = np.empty((N_SPOT, D), np.float32)
    for k in range(NCORES):
        out[k * SEG_PER_CORE : (k + 1) * SEG_PER_CORE] = res.results[k]["out"][
            :SEG_PER_CORE
        ]
    return out



# revision 2
# speedup vs baseline: 102.7091x; 102.7091x over previous
"""Trainium2 Bass kernel for AvgReadout-style segment mean + L2 normalize.

reference:
    vsum[i] = sum over edges e with src[e]==i of emb[dst[e]]
    deg[i]  = count of such edges (clamped to >=1)
    out     = l2_normalize(vsum / deg, eps=1e-12)

Key identity: l2_normalize(vsum/deg) == l2_normalize(vsum) whenever deg >= 1
(positive per-row scalar doesn't change direction), and for deg == 0 both are
exactly 0.  So the kernel only needs vsum, never deg.

Distribution: edges are sorted by src on host and sharded by src-range across
8 cores (12500 segments each).  Each core's output slice is disjoint, so no
collectives are needed.

Per core the segments are processed in 98 blocks of 128.  Edge rows are
fetched with dma_gather (int16 indices, so emb is addressed as 4 quarter
tables of 25000 rows); edges are grouped into cells (block, quarter), padded
to whole subtiles of 128 edges.  Cell capacities are maxed across cores so a
single compiled program serves all 8 cores.  Blocks are processed in
superblocks of 4; within a superblock the subtiles are quarter-major so each
dma_gather call (<=1024 indices; round-robined over 4 SWDGE queues) reads one
quarter table.

Per subtile a one-hot (edge -> segment-in-block) matrix built on DVE feeds a
PE matmul accumulating into the block's PSUM tile [128 segs, 128 feat].  The
one-hot build is batched per gather call in a (seg-major, subtile-minor)
layout so that every DVE operand has a packed innermost dimension, which
enables the DVE 2x 16-bit mode (the natural layout broadcasts the srcloc
operand with stride 0 innermost and runs at 1x).  The matmul lhsT reads the
one-hot with stride CALL_CAP over segment columns.  Pad edges carry an
out-of-range srcloc sentinel so their one-hot column is all zeros; their
gather index is 0 so the fetched row is always finite.

Epilogue per block: sum-of-squares (ACT Square+accum), sqrt, clamp 1e-12,
reciprocal, scale-copy, DMA out.

build_program(repeats=N) unrolls the whole body N times inside one NEFF;
hw_loop=True wraps it in a tc.For_i hardware loop instead (constant compile
time).  test.py uses that to amortize the multi-ms axon dispatch overhead out
of the per-iteration timing measurement.
"""

import numpy as np
from contextlib import ExitStack

N_SPOT = 100000
D = 128
P = 128
NCORES = 8
SEG_PER_CORE = 12500
NBLK = (SEG_PER_CORE + P - 1) // P  # 98
NQ = 4            # emb quarter tables (int16 index range)
QROWS = N_SPOT // NQ  # 25000
SB = 4            # blocks per superblock (PSUM tiles live concurrently)
CALL_CAP = 8      # subtiles per dma_gather call
NQUEUES = 4       # SWDGE queues to round-robin
RING = 16384      # dynamic DMA scratch bytes (1024 descriptor ring)
PAD_SENTINEL = 999.0


def compute_layout(capsub, call_cap=CALL_CAP):
    """capsub: [nblk, NQ] int array of per-cell subtile capacities."""
    capsub = np.asarray(capsub)
    nblk = capsub.shape[0]
    slot_block = []
    calls = []
    blk_slots = [[] for _ in range(nblk)]
    sb_list = []
    slot = 0
    for sb0 in range(0, nblk, SB):
        blocks = list(range(sb0, min(sb0 + SB, nblk)))
        call_lo = len(calls)
        for q in range(NQ):
            run = []  # slots of this (superblock, q) run
            for b in blocks:
                for _ in range(int(capsub[b, q])):
                    slot_block.append(b)
                    blk_slots[b].append(slot)
                    run.append(slot)
                    slot += 1
            for i in range(0, len(run), call_cap):
                chunk = run[i : i + call_cap]
                calls.append((q, chunk[0], len(chunk)))
        sb_list.append((blocks, (call_lo, len(calls))))
    return {
        "nslots": slot,
        "slot_block": slot_block,
        "calls": calls,
        "blk_slots": blk_slots,
        "sb_list": sb_list,
    }


def preprocess(emb, mask, call_cap=CALL_CAP):
    """Sort/shard/pad edges. Returns (in_maps, capsub, layout)."""
    qrows = QROWS
    emb = np.ascontiguousarray(np.asarray(emb, dtype=np.float32))
    emb16 = emb.astype(np.float16)
    mask = np.asarray(mask)
    src = mask[0].astype(np.int64, copy=False)
    dst = mask[1].astype(np.int64, copy=False)

    order = np.argsort(src, kind="stable")
    src_s = src[order].astype(np.int32)
    dst_s = dst[order].astype(np.int32)

    core_bounds = np.searchsorted(
        src_s, (SEG_PER_CORE * np.arange(NCORES + 1)).astype(np.int32)
    )

    percore = []
    cnts = np.zeros((NCORES, NBLK * NQ), np.int64)
    for k in range(NCORES):
        lo, hi = int(core_bounds[k]), int(core_bounds[k + 1])
        s = src_s[lo:hi] - SEG_PER_CORE * k
        d = dst_s[lo:hi]
        cell = (s >> 7) * NQ + d // qrows
        o = np.lexsort((d, cell))
        s, d, cell = s[o], d[o], cell[o]
        cnts[k] = np.bincount(cell, minlength=NBLK * NQ)
        percore.append((s, d, cell))

    capsub = (-(-cnts.max(axis=0) // P)).reshape(NBLK, NQ).astype(np.int64)
    layout = compute_layout(capsub, call_cap)
    nslots = layout["nslots"]

    cell_slot0 = np.zeros(NBLK * NQ, np.int64)
    slot = 0
    for sb0 in range(0, NBLK, SB):
        for q in range(NQ):
            for b in range(sb0, min(sb0 + SB, NBLK)):
                cell_slot0[b * NQ + q] = slot
                slot += int(capsub[b, q])
    assert slot == nslots

    # iota_rep[p, c*call_cap + t] = c  (one-hot build operand + lhsT layout)
    iota_rep = np.broadcast_to(
        np.repeat(np.arange(P), call_cap).astype(np.float16)[None, :],
        (P, P * call_cap),
    ).copy()

    in_maps = []
    for k in range(NCORES):
        s, d, cell = percore[k]
        cum = np.zeros(NBLK * NQ, np.int64)
        cc = cnts[k]
        cum[1:] = np.cumsum(cc)[:-1]
        rank = np.arange(len(s), dtype=np.int64) - cum[cell]
        pos = cell_slot0[cell] * P + rank  # global edge position

        srcloc = np.full(nslots * P, PAD_SENTINEL, np.float16)
        srcloc[pos] = (s & 127).astype(np.float16)
        dloc = np.zeros(nslots * P, np.int16)
        dloc[pos] = (d % qrows).astype(np.int16)

        # srcloc tile [p, slot] = value of edge (slot, p)
        srcloc_t = np.ascontiguousarray(srcloc.reshape(nslots, P).T)
        # idx16 [j%16, slot*8 + j//16] = dloc of edge (slot, j), replicated
        # across the 8 partition groups for the Q7 ucode.
        idx_blk = np.ascontiguousarray(dloc.reshape(nslots * 8, 16).T)
        idx16 = np.tile(idx_blk, (8, 1))
        in_maps.append(
            {"emb": emb16, "srcloc": srcloc_t, "dstidx": idx16, "iota": iota_rep}
        )
    return in_maps, capsub, layout


def build_program(capsub, layout, repeats=1, call_cap=CALL_CAP, ring=RING,
                  gbufs=24, obufs=24, hw_loop=False):
    import concourse.bass as bass
    import concourse.tile as tile
    from concourse import bacc, mybir

    qrows = QROWS
    nslots = layout["nslots"]
    calls = layout["calls"]
    blk_slots = layout["blk_slots"]
    sb_list = layout["sb_list"]
    d = D

    nc = bacc.Bacc(
        "TRN2", target_bir_lowering=False, debug=False,
        num_swdge_queues=NQUEUES, dynamic_dma_scratch_size=ring,
    )
    emb_t = nc.dram_tensor("emb", [N_SPOT, d], mybir.dt.float16, kind="ExternalInput")
    srcloc_t = nc.dram_tensor(
        "srcloc", [P, nslots], mybir.dt.float16, kind="ExternalInput"
    )
    dstidx_t = nc.dram_tensor(
        "dstidx", [P, nslots * 8], mybir.dt.int16, kind="ExternalInput"
    )
    iota_t = nc.dram_tensor("iota", [P, P * call_cap], mybir.dt.float16,
                            kind="ExternalInput")
    out_t = nc.dram_tensor(
        "out", [NBLK * P, d], mybir.dt.float32, kind="ExternalOutput"
    )

    slot_call = [None] * nslots
    for ci, (q, s0, nsub) in enumerate(calls):
        for t in range(nsub):
            slot_call[s0 + t] = (ci, t)

    with tile.TileContext(nc) as tc, ExitStack() as ctx:
        consts = ctx.enter_context(tc.tile_pool(name="consts", bufs=1))
        gpool = ctx.enter_context(tc.tile_pool(name="gather", bufs=gbufs))
        ohpool = ctx.enter_context(tc.tile_pool(name="onehot", bufs=obufs))
        spool = ctx.enter_context(tc.tile_pool(name="scratch", bufs=4))
        opool = ctx.enter_context(tc.tile_pool(name="outs", bufs=4))
        ppool = ctx.enter_context(tc.tile_pool(name="psum", bufs=8, space="PSUM"))

        srcloc_sb = consts.tile([P, nslots], mybir.dt.float16)
        nc.sync.dma_start(srcloc_sb[:], srcloc_t.ap())
        dstidx_sb = consts.tile([P, nslots * 8], mybir.dt.int16)
        nc.sync.dma_start(dstidx_sb[:], dstidx_t.ap())
        iota_sb = consts.tile([P, P * call_cap], mybir.dt.float16)
        nc.sync.dma_start(iota_sb[:], iota_t.ap())

        out_ap = out_t.ap()
        emb_ap = emb_t.ap()

        def emit_body():
            callno = 0
            for blocks, (clo, chi) in sb_list:
                gtiles = {}
                ohtiles = {}
                for ci in range(clo, chi):
                    q, s0, nsub = calls[ci]
                    gt = gpool.tile([P, call_cap * d], mybir.dt.float16, tag="gt")
                    nc.gpsimd.dma_gather(
                        out_ap=gt[:, : nsub * d].rearrange(
                            "p (c e) -> p c e", e=d
                        ),
                        in_ap=emb_ap[q * qrows : (q + 1) * qrows, :],
                        idxs_ap=dstidx_sb[:, s0 * 8 : (s0 + nsub) * 8],
                        num_idxs=nsub * P,
                        num_idxs_reg=nsub * P,
                        elem_size=d,
                        single_packet=False,
                        queue_num=callno % NQUEUES,
                    )
                    gtiles[ci] = gt
                    callno += 1
                    # batched one-hot build, (c-major, t-minor) layout:
                    # oh[p, c*call_cap + t] = (iota[c] == srcloc[p, s0+t]).
                    # innermost dim t is packed on all operands -> DVE 2x mode
                    oh = ohpool.tile([P, call_cap * P], mybir.dt.float16, tag="oh")
                    full = oh[:, :]
                    srl = srcloc_sb[:, s0 : s0 + nsub]
                    oh3 = bass.AP(
                        full.tensor, full.offset,
                        [full.ap[0], [call_cap, P], [1, nsub]],
                    )
                    iota_full = iota_sb[:, :]
                    iota_b = bass.AP(
                        iota_full.tensor, iota_full.offset,
                        [iota_full.ap[0], [call_cap, P], [1, nsub]],
                    )
                    srl_b = bass.AP(
                        srl.tensor, srl.offset, [srl.ap[0], [0, P], [1, nsub]]
                    )
                    nc.vector.tensor_tensor(
                        out=oh3, in0=iota_b, in1=srl_b,
                        op=mybir.AluOpType.is_equal,
                    )
                    ohtiles[ci] = oh
                for b in blocks:
                    slots = blk_slots[b]
                    if not slots:
                        ot = opool.tile([P, d], mybir.dt.float32)
                        nc.vector.memset(ot[:], 0.0)
                        nc.sync.dma_start(out_ap[b * P : (b + 1) * P, :], ot[:])
                        continue
                    ps = ppool.tile([P, d], mybir.dt.float32, space="PSUM")
                    for i, sl in enumerate(slots):
                        ci, t = slot_call[sl]
                        ohfull = ohtiles[ci][:, :]
                        lhsT = bass.AP(
                            ohfull.tensor, ohfull.offset + t,
                            [ohfull.ap[0], [call_cap, P]],
                        )
                        nc.tensor.matmul(
                            ps[:],
                            lhsT=lhsT,
                            rhs=gtiles[ci][:, t * d : (t + 1) * d],
                            start=(i == 0),
                            stop=(i == len(slots) - 1),
                        )
                    sq = spool.tile([P, d], mybir.dt.float32)
                    ss = spool.tile([P, 1], mybir.dt.float32)
                    nc.scalar.activation(
                        sq[:], ps[:], mybir.ActivationFunctionType.Square,
                        accum_out=ss[:],
                    )
                    nrm = spool.tile([P, 1], mybir.dt.float32)
                    nc.scalar.activation(
                        nrm[:], ss[:], mybir.ActivationFunctionType.Sqrt
                    )
                    nc.vector.tensor_scalar(
                        out=nrm[:], in0=nrm[:], scalar1=1e-12, scalar2=None,
                        op0=mybir.AluOpType.max,
                    )
                    nc.vector.reciprocal(nrm[:], nrm[:])
                    ot = opool.tile([P, d], mybir.dt.float32)
                    nc.scalar.activation(
                        ot[:], ps[:], mybir.ActivationFunctionType.Copy,
                        scale=nrm[:],
                    )
                    nc.sync.dma_start(out_ap[b * P : (b + 1) * P, :], ot[:])

        if hw_loop and repeats > 1:
            with tc.For_i(0, repeats) as _i:
                emit_body()
        else:
            for _rep in range(repeats):
                emit_body()

    nc.compile()
    return nc


_PROGRAM_CACHE = {}


def _get_program(capsub, layout, **kw):
    key = (capsub.tobytes(), tuple(sorted(kw.items())))
    if key not in _PROGRAM_CACHE:
        _PROGRAM_CACHE[key] = build_program(capsub, layout, **kw)
    return _PROGRAM_CACHE[key]


def kernel(**inputs):
    emb = inputs["emb"]
    mask = inputs["mask"]
    in_maps, capsub, layout = preprocess(emb, mask)
    nc = _get_program(capsub, layout)

    from concourse.bass_utils import run_bass_kernel_spmd

    res = run_bass_kernel_spmd(nc, in_maps, core_ids=list(range(NCORES)))
    out = np.empty((N_SPOT, D), np.float32)
    for k in range(NCORES):
        out[k * SEG_PER_CORE : (k + 1) * SEG_PER_CORE] = res.results[k]["out"][
            :SEG_PER_CORE
        ]
    return out


# revision 3
# speedup vs baseline: 106.4843x; 1.0368x over previous
"""Trainium2 Bass kernel for AvgReadout-style segment mean + L2 normalize.

reference:
    vsum[i] = sum over edges e with src[e]==i of emb[dst[e]]
    deg[i]  = count of such edges (clamped to >=1)
    out     = l2_normalize(vsum / deg, eps=1e-12)

Key identity: l2_normalize(vsum/deg) == l2_normalize(vsum) whenever deg >= 1
(positive per-row scalar doesn't change direction), and for deg == 0 both are
exactly 0.  So the kernel only needs vsum, never deg.

Distribution: edges are sorted by src on host and sharded by src-range across
8 cores (12500 segments each).  Each core's output slice is disjoint, so no
collectives are needed.

Per core the segments are processed in 98 blocks of 128.  Edge rows are
fetched with dma_gather (int16 indices, so emb is addressed as 4 quarter
tables of 25000 rows); edges are grouped into cells (block, quarter), padded
to whole subtiles of 128 edges.  Cell capacities are maxed across cores so a
single compiled program serves all 8 cores.  Blocks are processed in
superblocks of 4; within a superblock the subtiles are quarter-major so each
dma_gather call (<=1024 indices; round-robined over 4 SWDGE queues) reads one
quarter table.

Per subtile a one-hot (edge -> segment-in-block) matrix built on DVE feeds a
PE matmul accumulating into the block's PSUM tile [128 segs, 128 feat].  The
one-hot build is batched per gather call in a (seg-major, subtile-minor)
layout so that every DVE operand has a packed innermost dimension, which
enables the DVE 2x 16-bit mode (the natural layout broadcasts the srcloc
operand with stride 0 innermost and runs at 1x).  The matmul lhsT reads the
one-hot with stride CALL_CAP over segment columns.  Pad edges carry an
out-of-range srcloc sentinel so their one-hot column is all zeros; their
gather index is 0 so the fetched row is always finite.

Epilogue per block: sum-of-squares (ACT Square+accum), sqrt, clamp 1e-12,
reciprocal, scale-copy, DMA out.

build_program(repeats=N) unrolls the whole body N times inside one NEFF;
hw_loop=True wraps it in a tc.For_i hardware loop instead (constant compile
time).  test.py uses that to amortize the multi-ms axon dispatch overhead out
of the per-iteration timing measurement.
"""

import numpy as np
from contextlib import ExitStack

N_SPOT = 100000
D = 128
P = 128
NCORES = 8
SEG_PER_CORE = 12500
NBLK = (SEG_PER_CORE + P - 1) // P  # 98
NQ = 4            # emb quarter tables (int16 index range)
QROWS = N_SPOT // NQ  # 25000
SB = 4            # blocks per superblock (PSUM tiles live concurrently)
CALL_CAP = 8      # subtiles per dma_gather call
NQUEUES = 4       # SWDGE queues to round-robin
RING = 16384      # dynamic DMA scratch bytes (1024 descriptor ring)
PAD_SENTINEL = 999.0


def compute_layout(capsub, call_cap=CALL_CAP):
    """capsub: [nblk, NQ] int array of per-cell subtile capacities."""
    capsub = np.asarray(capsub)
    nblk = capsub.shape[0]
    slot_block = []
    calls = []
    blk_slots = [[] for _ in range(nblk)]
    sb_list = []
    slot = 0
    for sb0 in range(0, nblk, SB):
        blocks = list(range(sb0, min(sb0 + SB, nblk)))
        call_lo = len(calls)
        for q in range(NQ):
            run = []  # slots of this (superblock, q) run
            for b in blocks:
                for _ in range(int(capsub[b, q])):
                    slot_block.append(b)
                    blk_slots[b].append(slot)
                    run.append(slot)
                    slot += 1
            for i in range(0, len(run), call_cap):
                chunk = run[i : i + call_cap]
                calls.append((q, chunk[0], len(chunk)))
        sb_list.append((blocks, (call_lo, len(calls))))
    return {
        "nslots": slot,
        "slot_block": slot_block,
        "calls": calls,
        "blk_slots": blk_slots,
        "sb_list": sb_list,
    }


def preprocess(emb, mask, call_cap=CALL_CAP):
    """Sort/shard/pad edges. Returns (in_maps, capsub, layout)."""
    qrows = QROWS
    emb = np.ascontiguousarray(np.asarray(emb, dtype=np.float32))
    emb16 = emb.astype(np.float16)
    mask = np.asarray(mask)
    src = mask[0].astype(np.int64, copy=False)
    dst = mask[1].astype(np.int64, copy=False)

    order = np.argsort(src, kind="stable")
    src_s = src[order].astype(np.int32)
    dst_s = dst[order].astype(np.int32)

    core_bounds = np.searchsorted(
        src_s, (SEG_PER_CORE * np.arange(NCORES + 1)).astype(np.int32)
    )

    percore = []
    cnts = np.zeros((NCORES, NBLK * NQ), np.int64)
    for k in range(NCORES):
        lo, hi = int(core_bounds[k]), int(core_bounds[k + 1])
        s = src_s[lo:hi] - SEG_PER_CORE * k
        d = dst_s[lo:hi]
        cell = (s >> 7) * NQ + d // qrows
        o = np.lexsort((d, cell))
        s, d, cell = s[o], d[o], cell[o]
        cnts[k] = np.bincount(cell, minlength=NBLK * NQ)
        percore.append((s, d, cell))

    capsub = (-(-cnts.max(axis=0) // P)).reshape(NBLK, NQ).astype(np.int64)
    layout = compute_layout(capsub, call_cap)
    nslots = layout["nslots"]

    cell_slot0 = np.zeros(NBLK * NQ, np.int64)
    slot = 0
    for sb0 in range(0, NBLK, SB):
        for q in range(NQ):
            for b in range(sb0, min(sb0 + SB, NBLK)):
                cell_slot0[b * NQ + q] = slot
                slot += int(capsub[b, q])
    assert slot == nslots

    # iota_rep[p, c*call_cap + t] = c  (one-hot build operand + lhsT layout)
    iota_rep = np.broadcast_to(
        np.repeat(np.arange(P), call_cap).astype(np.float16)[None, :],
        (P, P * call_cap),
    ).copy()

    in_maps = []
    for k in range(NCORES):
        s, d, cell = percore[k]
        cum = np.zeros(NBLK * NQ, np.int64)
        cc = cnts[k]
        cum[1:] = np.cumsum(cc)[:-1]
        rank = np.arange(len(s), dtype=np.int64) - cum[cell]
        pos = cell_slot0[cell] * P + rank  # global edge position

        srcloc = np.full(nslots * P, PAD_SENTINEL, np.float16)
        srcloc[pos] = (s & 127).astype(np.float16)
        dloc = np.zeros(nslots * P, np.int16)
        dloc[pos] = (d % qrows).astype(np.int16)

        # srcloc tile [p, slot] = value of edge (slot, p)
        srcloc_t = np.ascontiguousarray(srcloc.reshape(nslots, P).T)
        # idx16 [j%16, slot*8 + j//16] = dloc of edge (slot, j), replicated
        # across the 8 partition groups for the Q7 ucode.
        idx_blk = np.ascontiguousarray(dloc.reshape(nslots * 8, 16).T)
        idx16 = np.tile(idx_blk, (8, 1))
        in_maps.append(
            {"emb": emb16, "srcloc": srcloc_t, "dstidx": idx16, "iota": iota_rep}
        )
    return in_maps, capsub, layout


def build_program(capsub, layout, repeats=1, call_cap=CALL_CAP, ring=RING,
                  gbufs=24, obufs=24, hw_loop=False):
    import concourse.bass as bass
    import concourse.tile as tile
    from concourse import bacc, mybir

    qrows = QROWS
    nslots = layout["nslots"]
    calls = layout["calls"]
    blk_slots = layout["blk_slots"]
    sb_list = layout["sb_list"]
    d = D

    nc = bacc.Bacc(
        "TRN2", target_bir_lowering=False, debug=False,
        num_swdge_queues=NQUEUES, dynamic_dma_scratch_size=ring,
    )
    emb_t = nc.dram_tensor("emb", [N_SPOT, d], mybir.dt.float16, kind="ExternalInput")
    srcloc_t = nc.dram_tensor(
        "srcloc", [P, nslots], mybir.dt.float16, kind="ExternalInput"
    )
    dstidx_t = nc.dram_tensor(
        "dstidx", [P, nslots * 8], mybir.dt.int16, kind="ExternalInput"
    )
    iota_t = nc.dram_tensor("iota", [P, P * call_cap], mybir.dt.float16,
                            kind="ExternalInput")
    out_t = nc.dram_tensor(
        "out", [NBLK * P, d], mybir.dt.float32, kind="ExternalOutput"
    )

    slot_call = [None] * nslots
    for ci, (q, s0, nsub) in enumerate(calls):
        for t in range(nsub):
            slot_call[s0 + t] = (ci, t)

    with tile.TileContext(nc) as tc, ExitStack() as ctx:
        consts = ctx.enter_context(tc.tile_pool(name="consts", bufs=1))
        gpool = ctx.enter_context(tc.tile_pool(name="gather", bufs=gbufs))
        ohpool = ctx.enter_context(tc.tile_pool(name="onehot", bufs=obufs))
        spool = ctx.enter_context(tc.tile_pool(name="scratch", bufs=4))
        opool = ctx.enter_context(tc.tile_pool(name="outs", bufs=4))
        ppool = ctx.enter_context(tc.tile_pool(name="psum", bufs=8, space="PSUM"))

        srcloc_sb = consts.tile([P, nslots], mybir.dt.float16)
        nc.sync.dma_start(srcloc_sb[:], srcloc_t.ap())
        dstidx_sb = consts.tile([P, nslots * 8], mybir.dt.int16)
        nc.sync.dma_start(dstidx_sb[:], dstidx_t.ap())
        iota_sb = consts.tile([P, P * call_cap], mybir.dt.float16)
        nc.sync.dma_start(iota_sb[:], iota_t.ap())

        out_ap = out_t.ap()
        emb_ap = emb_t.ap()

        def emit_body():
            callno = 0
            for blocks, (clo, chi) in sb_list:
                gtiles = {}
                ohtiles = {}
                for ci in range(clo, chi):
                    q, s0, nsub = calls[ci]
                    gt = gpool.tile([P, call_cap * d], mybir.dt.float16, tag="gt")
                    nc.gpsimd.dma_gather(
                        out_ap=gt[:, : nsub * d].rearrange(
                            "p (c e) -> p c e", e=d
                        ),
                        in_ap=emb_ap[q * qrows : (q + 1) * qrows, :],
                        idxs_ap=dstidx_sb[:, s0 * 8 : (s0 + nsub) * 8],
                        num_idxs=nsub * P,
                        num_idxs_reg=nsub * P,
                        elem_size=d,
                        single_packet=False,
                        queue_num=callno % NQUEUES,
                    )
                    gtiles[ci] = gt
                    callno += 1
                    # batched one-hot build, (c-major, t-minor) layout:
                    # oh[p, c*call_cap + t] = (iota[c] == srcloc[p, s0+t]).
                    # innermost dim t is packed on all operands -> DVE 2x mode
                    oh = ohpool.tile([P, call_cap * P], mybir.dt.float16, tag="oh")
                    full = oh[:, :]
                    srl = srcloc_sb[:, s0 : s0 + nsub]
                    oh3 = bass.AP(
                        full.tensor, full.offset,
                        [full.ap[0], [call_cap, P], [1, nsub]],
                    )
                    iota_full = iota_sb[:, :]
                    iota_b = bass.AP(
                        iota_full.tensor, iota_full.offset,
                        [iota_full.ap[0], [call_cap, P], [1, nsub]],
                    )
                    srl_b = bass.AP(
                        srl.tensor, srl.offset, [srl.ap[0], [0, P], [1, nsub]]
                    )
                    nc.vector.tensor_tensor(
                        out=oh3, in0=iota_b, in1=srl_b,
                        op=mybir.AluOpType.is_equal,
                    )
                    ohtiles[ci] = oh
                for b in blocks:
                    slots = blk_slots[b]
                    if not slots:
                        ot = opool.tile([P, d], mybir.dt.float32)
                        nc.vector.memset(ot[:], 0.0)
                        nc.sync.dma_start(out_ap[b * P : (b + 1) * P, :], ot[:])
                        continue
                    ps = ppool.tile([P, d], mybir.dt.float32, space="PSUM")
                    for i, sl in enumerate(slots):
                        ci, t = slot_call[sl]
                        ohfull = ohtiles[ci][:, :]
                        lhsT = bass.AP(
                            ohfull.tensor, ohfull.offset + t,
                            [ohfull.ap[0], [call_cap, P]],
                        )
                        nc.tensor.matmul(
                            ps[:],
                            lhsT=lhsT,
                            rhs=gtiles[ci][:, t * d : (t + 1) * d],
                            start=(i == 0),
                            stop=(i == len(slots) - 1),
                        )
                    sq = spool.tile([P, d], mybir.dt.float32)
                    ss = spool.tile([P, 1], mybir.dt.float32)
                    nc.scalar.activation(
                        sq[:], ps[:], mybir.ActivationFunctionType.Square,
                        accum_out=ss[:],
                    )
                    nrm = spool.tile([P, 1], mybir.dt.float32)
                    nc.scalar.activation(
                        nrm[:], ss[:], mybir.ActivationFunctionType.Sqrt
                    )
                    nc.vector.tensor_scalar(
                        out=nrm[:], in0=nrm[:], scalar1=1e-12, scalar2=None,
                        op0=mybir.AluOpType.max,
                    )
                    nc.vector.reciprocal(nrm[:], nrm[:])
                    ot = opool.tile([P, d], mybir.dt.float32)
                    nc.scalar.activation(
                        ot[:], ps[:], mybir.ActivationFunctionType.Copy,
                        scale=nrm[:],
                    )
                    nc.sync.dma_start(out_ap[b * P : (b + 1) * P, :], ot[:])

        if hw_loop and repeats > 1:
            # unroll UNROLL bodies per loop iteration: the For_i all-engine
            # barrier + semaphore reset drains the pipeline each iteration
            # (~100us), so amortize it over several kernel executions
            assert repeats % hw_loop == 0
            with tc.For_i(0, repeats // hw_loop) as _i:
                for _u in range(hw_loop):
                    emit_body()
        else:
            for _rep in range(repeats):
                emit_body()

    nc.compile()
    return nc


_PROGRAM_CACHE = {}


def _get_program(capsub, layout, **kw):
    key = (capsub.tobytes(), tuple(sorted(kw.items())))
    if key not in _PROGRAM_CACHE:
        _PROGRAM_CACHE[key] = build_program(capsub, layout, **kw)
    return _PROGRAM_CACHE[key]


def kernel(**inputs):
    emb = inputs["emb"]
    mask = inputs["mask"]
    in_maps, capsub, layout = preprocess(emb, mask)
    nc = _get_program(capsub, layout)

    from concourse.bass_utils import run_bass_kernel_spmd

    res = run_bass_kernel_spmd(nc, in_maps, core_ids=list(range(NCORES)))
    out = np.empty((N_SPOT, D), np.float32)
    for k in range(NCORES):
        out[k * SEG_PER_CORE : (k + 1) * SEG_PER_CORE] = res.results[k]["out"][
            :SEG_PER_CORE
        ]
    return out


# revision 5
# speedup vs baseline: 126.4358x; 1.1874x over previous
"""Trainium2 Bass kernel for AvgReadout-style segment mean + L2 normalize.

reference:
    vsum[i] = sum over edges e with src[e]==i of emb[dst[e]]
    deg[i]  = count of such edges (clamped to >=1)
    out     = l2_normalize(vsum / deg, eps=1e-12)

Key identity: l2_normalize(vsum/deg) == l2_normalize(vsum) whenever deg >= 1
(positive per-row scalar doesn't change direction), and for deg == 0 both are
exactly 0.  So the kernel only needs vsum, never deg.

Distribution: edges are sorted by src on host and sharded by src-range across
8 cores (12500 segments each).  Each core's output slice is disjoint, so no
collectives are needed.

Per core the 12500 segments form 98 blocks of 128, processed in superblocks
of SB=4 blocks (4 concurrent PSUM tiles).  Edge rows are fetched with
dma_gather (int16 indices force 4 quarter tables of 25000 emb rows).  Edges
are bucketed into cells (superblock, quarter) and padded to whole subtiles of
128 edges; cell capacities are maxed across cores so one compiled program
serves all 8 cores (measured padding ~6.5%; per-block cells would cost 25%).
The gather is HBM-random-access-bound (~2.4ns/row on HW), so total gathered
slot count is the dominant cost; pad indices are spread across the quarter
because repeated fetches of one row serialize on an HBM bank (measured 2.7x
slowdown when all indices equal).

Within a cell, edges are sorted by segment, so a subtile usually holds edges
of one block and spans two at block transitions.  Each slot carries a bitmask
of rel-blocks present (union across cores).  Per (gather call, rel-block) one
batched one-hot build on DVE compares srcloc (s mod 512) against an iota
slice offset by 128*rb; the layout is (seg-major, subtile-minor) so every
DVE operand has a packed innermost dim, enabling the DVE 2x 16-bit mode.
Edges of other blocks mismatch and contribute zero columns.  The PE matmul
for block sb0+rb accumulates lhsT = one-hot columns (stride CALL_CAP) x
rhs = gathered rows into the block's PSUM tile [128 segs, 128 feat].  Pad
edges carry an out-of-range srcloc sentinel (all-zero one-hot column); their
fetched rows are real emb rows, so everything stays finite.

Epilogue per block: sum-of-squares (ACT Square+accum), sqrt, clamp 1e-12,
reciprocal, scale-copy, DMA out.

build_program(repeats=N) unrolls the body N times in one NEFF; hw_loop=U
instead wraps N/U iterations of U unrolled bodies in a tc.For_i hardware
loop (constant compile time; the per-iteration all-engine barrier is
amortized over U bodies).  test.py uses that to amortize the ~60ms axon
dispatch floor out of the per-iteration timing.
"""

import numpy as np
from contextlib import ExitStack

N_SPOT = 100000
D = 128
P = 128
NCORES = 8
SEG_PER_CORE = 12500
NBLK = (SEG_PER_CORE + P - 1) // P  # 98
NQ = 4                 # emb quarter tables (int16 index range)
QROWS = N_SPOT // NQ   # 25000
SB = 4                 # blocks per superblock == blocks per cell
NSB = (NBLK + SB - 1) // SB  # 25
CALL_CAP = 8           # subtiles per dma_gather call
NQUEUES = 4            # SWDGE queues to round-robin
RING = 16384           # dynamic DMA scratch bytes (1024 descriptor ring)
PAD_SENTINEL = 3000.0  # outside [0, SB*P): pad edges match no one-hot column


def preprocess(emb, mask, call_cap=CALL_CAP):
    """Sort/shard/pad edges. Returns (in_maps, capsub, layout)."""
    qrows = QROWS
    emb = np.ascontiguousarray(np.asarray(emb, dtype=np.float32))
    emb16 = emb.astype(np.float16)
    mask = np.asarray(mask)
    src = mask[0].astype(np.int64, copy=False)
    dst = mask[1].astype(np.int64, copy=False)

    order = np.argsort(src, kind="stable")
    src_s = src[order].astype(np.int32)
    dst_s = dst[order].astype(np.int32)

    core_bounds = np.searchsorted(
        src_s, (SEG_PER_CORE * np.arange(NCORES + 1)).astype(np.int32)
    )

    ncell = NSB * NQ
    percore = []
    cnts = np.zeros((NCORES, ncell), np.int64)
    for k in range(NCORES):
        lo, hi = int(core_bounds[k]), int(core_bounds[k + 1])
        s = src_s[lo:hi] - SEG_PER_CORE * k
        d = dst_s[lo:hi]
        cell = (s >> 9) * NQ + d // qrows
        # sort by (cell, rel-block, d): rel-block grouping keeps subtile
        # spanning minimal; d-order within a block keeps the HBM gather
        # access pattern local (the gather is HBM-random-bound)
        o = np.lexsort((d, s >> 7, cell))
        s, d, cell = s[o], d[o], cell[o]
        cnts[k] = np.bincount(cell, minlength=ncell)
        percore.append((s, d, cell))

    capsub = (-(-cnts.max(axis=0) // P)).astype(np.int64)  # [ncell] subtiles
    nslots = int(capsub.sum())

    cell_slot0 = np.zeros(ncell, np.int64)
    cell_slot0[1:] = np.cumsum(capsub)[:-1]
    cell_base = cell_slot0 * P

    slot_mask = np.zeros(nslots, np.int64)  # rel-block bitmask, cross-core union

    # iota[p, (c*call_cap) + t] = c for c in [0, SB*P)
    iota512 = np.broadcast_to(
        np.repeat(np.arange(SB * P), call_cap).astype(np.float16)[None, :],
        (P, SB * P * call_cap),
    ).copy()

    # spread pad gather indices across the quarter: repeated fetches of a
    # single row serialize on an HBM bank (measured 2.7x slowdown)
    pad_spread = ((np.arange(nslots * P, dtype=np.int64) * 97) % qrows).astype(
        np.int16
    )

    in_maps = []
    for k in range(NCORES):
        s, d, cell = percore[k]
        cum = np.zeros(ncell, np.int64)
        cc = cnts[k]
        cum[1:] = np.cumsum(cc)[:-1]
        rank = np.arange(len(s), dtype=np.int64) - cum[cell]
        pos = cell_base[cell] + rank

        slot_g = cell_slot0[cell] + (rank >> 7)
        rb = (s >> 7) & (SB - 1)
        np.bitwise_or.at(slot_mask, slot_g, 1 << rb)

        srcloc = np.full(nslots * P, PAD_SENTINEL, np.float16)
        srcloc[pos] = (s & (SB * P - 1)).astype(np.float16)
        dloc = pad_spread.copy()
        dloc[pos] = (d % qrows).astype(np.int16)

        srcloc_t = np.ascontiguousarray(srcloc.reshape(nslots, P).T)
        # idx16 [j%16, slot*8 + j//16] = dloc of edge (slot, j), replicated
        # across the 8 partition groups for the Q7 ucode.
        idx_blk = np.ascontiguousarray(dloc.reshape(nslots * 8, 16).T)
        idx16 = np.tile(idx_blk, (8, 1))
        in_maps.append(
            {"emb": emb16, "srcloc": srcloc_t, "dstidx": idx16, "iota": iota512}
        )

    # layout: gather calls + per-call rb runs + per-block matmul lists
    calls = []           # (q, slot0, nsub)
    call_rb_runs = []    # per call: [(rb, t_lo, t_len)]
    blk_matmuls = [[] for _ in range(NBLK)]  # (call_idx, t, rb)
    sb_list = []         # (blocks, (call_lo, call_hi))
    for isb in range(NSB):
        blocks = list(range(isb * SB, min((isb + 1) * SB, NBLK)))
        call_lo = len(calls)
        for q in range(NQ):
            c = isb * NQ + q
            s0c = int(cell_slot0[c])
            cap = int(capsub[c])
            for i in range(0, cap, call_cap):
                nsub = min(call_cap, cap - i)
                s0 = s0c + i
                ci = len(calls)
                calls.append((q, s0, nsub))
                runs = []
                for rb in range(SB):
                    ts = [t for t in range(nsub)
                          if slot_mask[s0 + t] & (1 << rb)]
                    if not ts:
                        continue
                    assert ts == list(range(ts[0], ts[0] + len(ts))), (
                        "rb slots not contiguous in call"
                    )
                    runs.append((rb, ts[0], len(ts)))
                    b = isb * SB + rb
                    for t in ts:
                        blk_matmuls[b].append((ci, t, rb))
                call_rb_runs.append(runs)
        sb_list.append((blocks, (call_lo, len(calls))))

    layout = {
        "nslots": nslots,
        "calls": calls,
        "call_rb_runs": call_rb_runs,
        "blk_matmuls": blk_matmuls,
        "sb_list": sb_list,
    }
    return in_maps, capsub, layout


def build_program(capsub, layout, repeats=1, call_cap=CALL_CAP, ring=RING,
                  gbufs=24, obufs=24, hw_loop=False):
    import concourse.bass as bass
    import concourse.tile as tile
    from concourse import bacc, mybir

    qrows = QROWS
    nslots = layout["nslots"]
    calls = layout["calls"]
    call_rb_runs = layout["call_rb_runs"]
    blk_matmuls = layout["blk_matmuls"]
    sb_list = layout["sb_list"]
    d = D

    nc = bacc.Bacc(
        "TRN2", target_bir_lowering=False, debug=False,
        num_swdge_queues=NQUEUES, dynamic_dma_scratch_size=ring,
    )
    emb_t = nc.dram_tensor("emb", [N_SPOT, d], mybir.dt.float16, kind="ExternalInput")
    srcloc_t = nc.dram_tensor(
        "srcloc", [P, nslots], mybir.dt.float16, kind="ExternalInput"
    )
    dstidx_t = nc.dram_tensor(
        "dstidx", [P, nslots * 8], mybir.dt.int16, kind="ExternalInput"
    )
    iota_t = nc.dram_tensor("iota", [P, SB * P * call_cap], mybir.dt.float16,
                            kind="ExternalInput")
    out_t = nc.dram_tensor(
        "out", [NBLK * P, d], mybir.dt.float32, kind="ExternalOutput"
    )

    with tile.TileContext(nc) as tc, ExitStack() as ctx:
        consts = ctx.enter_context(tc.tile_pool(name="consts", bufs=1))
        gpool = ctx.enter_context(tc.tile_pool(name="gather", bufs=gbufs))
        ohpool = ctx.enter_context(tc.tile_pool(name="onehot", bufs=obufs))
        spool = ctx.enter_context(tc.tile_pool(name="scratch", bufs=4))
        opool = ctx.enter_context(tc.tile_pool(name="outs", bufs=4))
        ppool = ctx.enter_context(tc.tile_pool(name="psum", bufs=8, space="PSUM"))

        srcloc_sb = consts.tile([P, nslots], mybir.dt.float16)
        nc.sync.dma_start(srcloc_sb[:], srcloc_t.ap())
        dstidx_sb = consts.tile([P, nslots * 8], mybir.dt.int16)
        nc.sync.dma_start(dstidx_sb[:], dstidx_t.ap())
        iota_sb = consts.tile([P, SB * P * call_cap], mybir.dt.float16)
        nc.sync.dma_start(iota_sb[:], iota_t.ap())

        out_ap = out_t.ap()
        emb_ap = emb_t.ap()

        def emit_body():
            callno = 0
            for blocks, (clo, chi) in sb_list:
                gtiles = {}
                ohtiles = {}
                for ci in range(clo, chi):
                    q, s0, nsub = calls[ci]
                    gt = gpool.tile([P, call_cap * d], mybir.dt.float16, tag="gt")
                    nc.gpsimd.dma_gather(
                        out_ap=gt[:, : nsub * d].rearrange(
                            "p (c e) -> p c e", e=d
                        ),
                        in_ap=emb_ap[q * qrows : (q + 1) * qrows, :],
                        idxs_ap=dstidx_sb[:, s0 * 8 : (s0 + nsub) * 8],
                        num_idxs=nsub * P,
                        num_idxs_reg=nsub * P,
                        elem_size=d,
                        single_packet=False,
                        queue_num=callno % NQUEUES,
                    )
                    gtiles[ci] = gt
                    callno += 1
                    iota_full = iota_sb[:, :]
                    srl0 = srcloc_sb[:, s0 : s0 + nsub]
                    for rb, t_lo, t_len in call_rb_runs[ci]:
                        oh = ohpool.tile([P, call_cap * P], mybir.dt.float16,
                                         tag="oh")
                        full = oh[:, :]
                        oh3 = bass.AP(
                            full.tensor, full.offset + t_lo,
                            [full.ap[0], [call_cap, P], [1, t_len]],
                        )
                        iota_b = bass.AP(
                            iota_full.tensor,
                            iota_full.offset + rb * P * call_cap + t_lo,
                            [iota_full.ap[0], [call_cap, P], [1, t_len]],
                        )
                        srl_b = bass.AP(
                            srl0.tensor, srl0.offset + t_lo,
                            [srl0.ap[0], [0, P], [1, t_len]],
                        )
                        nc.vector.tensor_tensor(
                            out=oh3, in0=iota_b, in1=srl_b,
                            op=mybir.AluOpType.is_equal,
                        )
                        ohtiles[(ci, rb)] = oh
                for b in blocks:
                    mms = blk_matmuls[b]
                    if not mms:
                        ot = opool.tile([P, d], mybir.dt.float32)
                        nc.vector.memset(ot[:], 0.0)
                        nc.sync.dma_start(out_ap[b * P : (b + 1) * P, :], ot[:])
                        continue
                    ps = ppool.tile([P, d], mybir.dt.float32, space="PSUM")
                    for i, (ci, t, rb) in enumerate(mms):
                        ohfull = ohtiles[(ci, rb)][:, :]
                        lhsT = bass.AP(
                            ohfull.tensor, ohfull.offset + t,
                            [ohfull.ap[0], [call_cap, P]],
                        )
                        nc.tensor.matmul(
                            ps[:],
                            lhsT=lhsT,
                            rhs=gtiles[ci][:, t * d : (t + 1) * d],
                            start=(i == 0),
                            stop=(i == len(mms) - 1),
                        )
                    sq = spool.tile([P, d], mybir.dt.float32)
                    ss = spool.tile([P, 1], mybir.dt.float32)
                    nc.scalar.activation(
                        sq[:], ps[:], mybir.ActivationFunctionType.Square,
                        accum_out=ss[:],
                    )
                    nrm = spool.tile([P, 1], mybir.dt.float32)
                    nc.scalar.activation(
                        nrm[:], ss[:], mybir.ActivationFunctionType.Sqrt
                    )
                    nc.vector.tensor_scalar(
                        out=nrm[:], in0=nrm[:], scalar1=1e-12, scalar2=None,
                        op0=mybir.AluOpType.max,
                    )
                    nc.vector.reciprocal(nrm[:], nrm[:])
                    ot = opool.tile([P, d], mybir.dt.float32)
                    nc.scalar.activation(
                        ot[:], ps[:], mybir.ActivationFunctionType.Copy,
                        scale=nrm[:],
                    )
                    nc.sync.dma_start(out_ap[b * P : (b + 1) * P, :], ot[:])

        if hw_loop and repeats > 1:
            assert repeats % hw_loop == 0
            with tc.For_i(0, repeats // hw_loop) as _i:
                for _u in range(hw_loop):
                    emit_body()
        else:
            for _rep in range(repeats):
                emit_body()

    nc.compile()
    return nc


_PROGRAM_CACHE = {}


def _get_program(capsub, layout, **kw):
    key = (capsub.tobytes(), tuple(sorted(kw.items())))
    if key not in _PROGRAM_CACHE:
        _PROGRAM_CACHE[key] = build_program(capsub, layout, **kw)
    return _PROGRAM_CACHE[key]


def kernel(**inputs):
    emb = inputs["emb"]
    mask = inputs["mask"]
    in_maps, capsub, layout = preprocess(emb, mask)
    nc = _get_program(capsub, layout)

    from concourse.bass_utils import run_bass_kernel_spmd

    res = run_bass_kernel_spmd(nc, in_maps, core_ids=list(range(NCORES)))
    out = np.empty((N_SPOT, D), np.float32)
    for k in range(NCORES):
        out[k * SEG_PER_CORE : (k + 1) * SEG_PER_CORE] = res.results[k]["out"][
            :SEG_PER_CORE
        ]
    return out


# revision 6
# speedup vs baseline: 126.7451x; 1.0024x over previous
"""Trainium2 Bass kernel for AvgReadout-style segment mean + L2 normalize.

reference:
    vsum[i] = sum over edges e with src[e]==i of emb[dst[e]]
    deg[i]  = count of such edges (clamped to >=1)
    out     = l2_normalize(vsum / deg, eps=1e-12)

Key identity: l2_normalize(vsum/deg) == l2_normalize(vsum) whenever deg >= 1
(positive per-row scalar doesn't change direction), and for deg == 0 both are
exactly 0.  So the kernel only needs vsum, never deg.

Distribution: edges are sorted by src on host and sharded by src-range across
8 cores (12500 segments each).  Each core's output slice is disjoint, so no
collectives are needed.

Per core the 12500 segments form 98 blocks of 128, processed in superblocks
of SB=4 blocks (4 concurrent PSUM tiles).  Edge rows are fetched with
dma_gather (int16 indices force 4 quarter tables of 25000 emb rows).  Edges
are bucketed into cells (superblock, quarter) and padded to whole subtiles of
128 edges; cell capacities are maxed across cores so one compiled program
serves all 8 cores (measured padding ~6.5%; per-block cells would cost 25%).
The gather is HBM-random-access-bound (~2.4ns/row on HW), so total gathered
slot count is the dominant cost; pad indices are spread across the quarter
because repeated fetches of one row serialize on an HBM bank (measured 2.7x
slowdown when all indices equal).

Within a cell, edges are sorted by segment, so a subtile usually holds edges
of one block and spans two at block transitions.  Each slot carries a bitmask
of rel-blocks present (union across cores).  Per (gather call, rel-block) one
batched one-hot build on DVE compares srcloc (s mod 512) against an iota
slice offset by 128*rb; the layout is (seg-major, subtile-minor) so every
DVE operand has a packed innermost dim, enabling the DVE 2x 16-bit mode.
Edges of other blocks mismatch and contribute zero columns.  The PE matmul
for block sb0+rb accumulates lhsT = one-hot columns (stride CALL_CAP) x
rhs = gathered rows into the block's PSUM tile [128 segs, 128 feat].  Pad
edges carry an out-of-range srcloc sentinel (all-zero one-hot column); their
fetched rows are real emb rows, so everything stays finite.

Epilogue per block: sum-of-squares (ACT Square+accum), sqrt, clamp 1e-12,
reciprocal, scale-copy, DMA out.

build_program(repeats=N) unrolls the body N times in one NEFF; hw_loop=U
instead wraps N/U iterations of U unrolled bodies in a tc.For_i hardware
loop (constant compile time; the per-iteration all-engine barrier is
amortized over U bodies).  test.py uses that to amortize the ~60ms axon
dispatch floor out of the per-iteration timing.
"""

import numpy as np
from contextlib import ExitStack

N_SPOT = 100000
D = 128
P = 128
NCORES = 8
SEG_PER_CORE = 12500
NBLK = (SEG_PER_CORE + P - 1) // P  # 98
NQ = 4                 # emb quarter tables (int16 index range)
QROWS = N_SPOT // NQ   # 25000
SB = 4                 # blocks per superblock == blocks per cell
NSB = (NBLK + SB - 1) // SB  # 25
CALL_CAP = 8           # subtiles per dma_gather call
NQUEUES = 4            # SWDGE queues to round-robin
RING = 16384           # dynamic DMA scratch bytes (1024 descriptor ring)
PAD_SENTINEL = 3000.0  # outside [0, SB*P): pad edges match no one-hot column


def preprocess(emb, mask, call_cap=CALL_CAP):
    """Sort/shard/pad edges. Returns (in_maps, capsub, layout)."""
    qrows = QROWS
    emb = np.ascontiguousarray(np.asarray(emb, dtype=np.float32))
    emb16 = emb.astype(np.float16)
    mask = np.asarray(mask)
    src = mask[0].astype(np.int64, copy=False)
    dst = mask[1].astype(np.int64, copy=False)

    order = np.argsort(src, kind="stable")
    src_s = src[order].astype(np.int32)
    dst_s = dst[order].astype(np.int32)

    core_bounds = np.searchsorted(
        src_s, (SEG_PER_CORE * np.arange(NCORES + 1)).astype(np.int32)
    )

    ncell = NSB * NQ
    percore = []
    cnts = np.zeros((NCORES, ncell), np.int64)
    for k in range(NCORES):
        lo, hi = int(core_bounds[k]), int(core_bounds[k + 1])
        s = src_s[lo:hi] - SEG_PER_CORE * k
        d = dst_s[lo:hi]
        cell = (s >> 9) * NQ + d // qrows
        # sort by (cell, rel-block, d): rel-block grouping keeps subtile
        # spanning minimal; d-order within a block keeps the HBM gather
        # access pattern local (the gather is HBM-random-bound)
        o = np.lexsort((d, s >> 7, cell))
        s, d, cell = s[o], d[o], cell[o]
        cnts[k] = np.bincount(cell, minlength=ncell)
        percore.append((s, d, cell))

    capsub = (-(-cnts.max(axis=0) // P)).astype(np.int64)  # [ncell] subtiles
    nslots = int(capsub.sum())

    cell_slot0 = np.zeros(ncell, np.int64)
    cell_slot0[1:] = np.cumsum(capsub)[:-1]
    cell_base = cell_slot0 * P

    slot_mask = np.zeros(nslots, np.int64)  # rel-block bitmask, cross-core union

    # iota[p, (c*call_cap) + t] = c for c in [0, SB*P)
    iota512 = np.broadcast_to(
        np.repeat(np.arange(SB * P), call_cap).astype(np.float16)[None, :],
        (P, SB * P * call_cap),
    ).copy()

    # spread pad gather indices across the quarter: repeated fetches of a
    # single row serialize on an HBM bank (measured 2.7x slowdown)
    pad_spread = ((np.arange(nslots * P, dtype=np.int64) * 97) % qrows).astype(
        np.int16
    )

    in_maps = []
    for k in range(NCORES):
        s, d, cell = percore[k]
        cum = np.zeros(ncell, np.int64)
        cc = cnts[k]
        cum[1:] = np.cumsum(cc)[:-1]
        rank = np.arange(len(s), dtype=np.int64) - cum[cell]
        pos = cell_base[cell] + rank

        slot_g = cell_slot0[cell] + (rank >> 7)
        rb = (s >> 7) & (SB - 1)
        np.bitwise_or.at(slot_mask, slot_g, 1 << rb)

        srcloc = np.full(nslots * P, PAD_SENTINEL, np.float16)
        srcloc[pos] = (s & (SB * P - 1)).astype(np.float16)
        dloc = pad_spread.copy()
        dloc[pos] = (d % qrows).astype(np.int16)

        srcloc_t = np.ascontiguousarray(srcloc.reshape(nslots, P).T)
        # idx16 [j%16, slot*8 + j//16] = dloc of edge (slot, j), replicated
        # across the 8 partition groups for the Q7 ucode.
        idx_blk = np.ascontiguousarray(dloc.reshape(nslots * 8, 16).T)
        idx16 = np.tile(idx_blk, (8, 1))
        in_maps.append(
            {"emb": emb16, "srcloc": srcloc_t, "dstidx": idx16, "iota": iota512}
        )

    # layout: gather calls + per-call rb runs + per-block matmul lists
    calls = []           # (q, slot0, nsub)
    call_rb_runs = []    # per call: [(rb, t_lo, t_len)]
    blk_matmuls = [[] for _ in range(NBLK)]  # (call_idx, t, rb)
    sb_list = []         # (blocks, (call_lo, call_hi))
    for isb in range(NSB):
        blocks = list(range(isb * SB, min((isb + 1) * SB, NBLK)))
        call_lo = len(calls)
        for q in range(NQ):
            c = isb * NQ + q
            s0c = int(cell_slot0[c])
            cap = int(capsub[c])
            for i in range(0, cap, call_cap):
                nsub = min(call_cap, cap - i)
                s0 = s0c + i
                ci = len(calls)
                calls.append((q, s0, nsub))
                runs = []
                for rb in range(SB):
                    ts = [t for t in range(nsub)
                          if slot_mask[s0 + t] & (1 << rb)]
                    if not ts:
                        continue
                    # split into contiguous stretches (cross-core union of
                    # per-core contiguous ranges can, in principle, have gaps)
                    t_lo = ts[0]
                    prev = ts[0]
                    for t in ts[1:] + [None]:
                        if t is not None and t == prev + 1:
                            prev = t
                            continue
                        runs.append((rb, t_lo, prev - t_lo + 1))
                        if t is not None:
                            t_lo = prev = t
                    b = isb * SB + rb
                    for t in ts:
                        blk_matmuls[b].append((ci, t, rb))
                call_rb_runs.append(runs)
        sb_list.append((blocks, (call_lo, len(calls))))

    layout = {
        "nslots": nslots,
        "calls": calls,
        "call_rb_runs": call_rb_runs,
        "blk_matmuls": blk_matmuls,
        "sb_list": sb_list,
    }
    return in_maps, capsub, layout


def build_program(capsub, layout, repeats=1, call_cap=CALL_CAP, ring=RING,
                  gbufs=24, obufs=24, hw_loop=False):
    import concourse.bass as bass
    import concourse.tile as tile
    from concourse import bacc, mybir

    qrows = QROWS
    nslots = layout["nslots"]
    calls = layout["calls"]
    call_rb_runs = layout["call_rb_runs"]
    blk_matmuls = layout["blk_matmuls"]
    sb_list = layout["sb_list"]
    d = D

    nc = bacc.Bacc(
        "TRN2", target_bir_lowering=False, debug=False,
        num_swdge_queues=NQUEUES, dynamic_dma_scratch_size=ring,
    )
    emb_t = nc.dram_tensor("emb", [N_SPOT, d], mybir.dt.float16, kind="ExternalInput")
    srcloc_t = nc.dram_tensor(
        "srcloc", [P, nslots], mybir.dt.float16, kind="ExternalInput"
    )
    dstidx_t = nc.dram_tensor(
        "dstidx", [P, nslots * 8], mybir.dt.int16, kind="ExternalInput"
    )
    iota_t = nc.dram_tensor("iota", [P, SB * P * call_cap], mybir.dt.float16,
                            kind="ExternalInput")
    out_t = nc.dram_tensor(
        "out", [NBLK * P, d], mybir.dt.float32, kind="ExternalOutput"
    )

    with tile.TileContext(nc) as tc, ExitStack() as ctx:
        consts = ctx.enter_context(tc.tile_pool(name="consts", bufs=1))
        gpool = ctx.enter_context(tc.tile_pool(name="gather", bufs=gbufs))
        ohpool = ctx.enter_context(tc.tile_pool(name="onehot", bufs=obufs))
        spool = ctx.enter_context(tc.tile_pool(name="scratch", bufs=4))
        opool = ctx.enter_context(tc.tile_pool(name="outs", bufs=4))
        ppool = ctx.enter_context(tc.tile_pool(name="psum", bufs=8, space="PSUM"))

        srcloc_sb = consts.tile([P, nslots], mybir.dt.float16)
        nc.sync.dma_start(srcloc_sb[:], srcloc_t.ap())
        dstidx_sb = consts.tile([P, nslots * 8], mybir.dt.int16)
        nc.sync.dma_start(dstidx_sb[:], dstidx_t.ap())
        iota_sb = consts.tile([P, SB * P * call_cap], mybir.dt.float16)
        nc.sync.dma_start(iota_sb[:], iota_t.ap())

        out_ap = out_t.ap()
        emb_ap = emb_t.ap()

        def emit_body():
            callno = 0
            for blocks, (clo, chi) in sb_list:
                gtiles = {}
                ohtiles = {}
                for ci in range(clo, chi):
                    q, s0, nsub = calls[ci]
                    gt = gpool.tile([P, call_cap * d], mybir.dt.float16, tag="gt")
                    nc.gpsimd.dma_gather(
                        out_ap=gt[:, : nsub * d].rearrange(
                            "p (c e) -> p c e", e=d
                        ),
                        in_ap=emb_ap[q * qrows : (q + 1) * qrows, :],
                        idxs_ap=dstidx_sb[:, s0 * 8 : (s0 + nsub) * 8],
                        num_idxs=nsub * P,
                        num_idxs_reg=nsub * P,
                        elem_size=d,
                        single_packet=False,
                        queue_num=callno % NQUEUES,
                    )
                    gtiles[ci] = gt
                    callno += 1
                    iota_full = iota_sb[:, :]
                    srl0 = srcloc_sb[:, s0 : s0 + nsub]
                    for rb, t_lo, t_len in call_rb_runs[ci]:
                        oh = ohpool.tile([P, call_cap * P], mybir.dt.float16,
                                         tag="oh")
                        full = oh[:, :]
                        oh3 = bass.AP(
                            full.tensor, full.offset + t_lo,
                            [full.ap[0], [call_cap, P], [1, t_len]],
                        )
                        iota_b = bass.AP(
                            iota_full.tensor,
                            iota_full.offset + rb * P * call_cap + t_lo,
                            [iota_full.ap[0], [call_cap, P], [1, t_len]],
                        )
                        srl_b = bass.AP(
                            srl0.tensor, srl0.offset + t_lo,
                            [srl0.ap[0], [0, P], [1, t_len]],
                        )
                        nc.vector.tensor_tensor(
                            out=oh3, in0=iota_b, in1=srl_b,
                            op=mybir.AluOpType.is_equal,
                        )
                        ohtiles[(ci, rb)] = oh
                for b in blocks:
                    mms = blk_matmuls[b]
                    if not mms:
                        ot = opool.tile([P, d], mybir.dt.float32)
                        nc.vector.memset(ot[:], 0.0)
                        nc.sync.dma_start(out_ap[b * P : (b + 1) * P, :], ot[:])
                        continue
                    ps = ppool.tile([P, d], mybir.dt.float32, space="PSUM")
                    for i, (ci, t, rb) in enumerate(mms):
                        ohfull = ohtiles[(ci, rb)][:, :]
                        lhsT = bass.AP(
                            ohfull.tensor, ohfull.offset + t,
                            [ohfull.ap[0], [call_cap, P]],
                        )
                        nc.tensor.matmul(
                            ps[:],
                            lhsT=lhsT,
                            rhs=gtiles[ci][:, t * d : (t + 1) * d],
                            start=(i == 0),
                            stop=(i == len(mms) - 1),
                        )
                    sq = spool.tile([P, d], mybir.dt.float32)
                    ss = spool.tile([P, 1], mybir.dt.float32)
                    nc.scalar.activation(
                        sq[:], ps[:], mybir.ActivationFunctionType.Square,
                        accum_out=ss[:],
                    )
                    nrm = spool.tile([P, 1], mybir.dt.float32)
                    nc.scalar.activation(
                        nrm[:], ss[:], mybir.ActivationFunctionType.Sqrt
                    )
                    nc.vector.tensor_scalar(
                        out=nrm[:], in0=nrm[:], scalar1=1e-12, scalar2=None,
                        op0=mybir.AluOpType.max,
                    )
                    nc.vector.reciprocal(nrm[:], nrm[:])
                    ot = opool.tile([P, d], mybir.dt.float32)
                    nc.scalar.activation(
                        ot[:], ps[:], mybir.ActivationFunctionType.Copy,
                        scale=nrm[:],
                    )
                    nc.sync.dma_start(out_ap[b * P : (b + 1) * P, :], ot[:])

        if hw_loop and repeats > 1:
            assert repeats % hw_loop == 0
            with tc.For_i(0, repeats // hw_loop) as _i:
                for _u in range(hw_loop):
                    emit_body()
        else:
            for _rep in range(repeats):
                emit_body()

    nc.compile()
    return nc


_PROGRAM_CACHE = {}


def _get_program(capsub, layout, **kw):
    key = (capsub.tobytes(), tuple(sorted(kw.items())))
    if key not in _PROGRAM_CACHE:
        _PROGRAM_CACHE[key] = build_program(capsub, layout, **kw)
    return _PROGRAM_CACHE[key]


def kernel(**inputs):
    emb = inputs["emb"]
    mask = inputs["mask"]
    in_maps, capsub, layout = preprocess(emb, mask)
    nc = _get_program(capsub, layout)

    from concourse.bass_utils import run_bass_kernel_spmd

    res = run_bass_kernel_spmd(nc, in_maps, core_ids=list(range(NCORES)))
    out = np.empty((N_SPOT, D), np.float32)
    for k in range(NCORES):
        out[k * SEG_PER_CORE : (k + 1) * SEG_PER_CORE] = res.results[k]["out"][
            :SEG_PER_CORE
        ]
    return out


# revision 7
# speedup vs baseline: 127.6077x; 1.0068x over previous
"""Trainium2 Bass kernel for AvgReadout-style segment mean + L2 normalize.

reference:
    vsum[i] = sum over edges e with src[e]==i of emb[dst[e]]
    deg[i]  = count of such edges (clamped to >=1)
    out     = l2_normalize(vsum / deg, eps=1e-12)

Key identity: l2_normalize(vsum/deg) == l2_normalize(vsum) whenever deg >= 1
(positive per-row scalar doesn't change direction), and for deg == 0 both are
exactly 0.  So the kernel only needs vsum, never deg.

Distribution: edges are sorted by src on host and sharded by src-range across
8 cores (12500 segments each).  Each core's output slice is disjoint, so no
collectives are needed.

Per core the 12500 segments form 98 blocks of 128, processed in superblocks
of SB=4 blocks (4 concurrent PSUM tiles).  Edge rows are fetched with
dma_gather (int16 indices force 4 quarter tables of 25000 emb rows).  Edges
are bucketed into cells (superblock, quarter) and padded to whole subtiles of
128 edges; cell capacities are maxed across cores so one compiled program
serves all 8 cores (measured padding ~6.5%; per-block cells would cost 25%).
The gather is HBM-random-access-bound (~2.4ns/row on HW), so total gathered
slot count is the dominant cost; pad indices are spread across the quarter
because repeated fetches of one row serialize on an HBM bank (measured 2.7x
slowdown when all indices equal).

Within a cell, edges are sorted by segment, so a subtile usually holds edges
of one block and spans two at block transitions.  Each slot carries a bitmask
of rel-blocks present (union across cores).  Per (gather call, rel-block) one
batched one-hot build on DVE compares srcloc (s mod 512) against an iota
slice offset by 128*rb; the layout is (seg-major, subtile-minor) so every
DVE operand has a packed innermost dim, enabling the DVE 2x 16-bit mode.
Edges of other blocks mismatch and contribute zero columns.  The PE matmul
for block sb0+rb accumulates lhsT = one-hot columns (stride CALL_CAP) x
rhs = gathered rows into the block's PSUM tile [128 segs, 128 feat].  Pad
edges carry an out-of-range srcloc sentinel (all-zero one-hot column); their
fetched rows are real emb rows, so everything stays finite.

Epilogue per block: sum-of-squares (ACT Square+accum), sqrt, clamp 1e-12,
reciprocal, scale-copy, DMA out.

build_program(repeats=N) unrolls the body N times in one NEFF; hw_loop=U
instead wraps N/U iterations of U unrolled bodies in a tc.For_i hardware
loop (constant compile time; the per-iteration all-engine barrier is
amortized over U bodies).  test.py uses that to amortize the ~60ms axon
dispatch floor out of the per-iteration timing.
"""

import numpy as np
from contextlib import ExitStack

N_SPOT = 100000
D = 128
P = 128
NCORES = 8
SEG_PER_CORE = 12500
NBLK = (SEG_PER_CORE + P - 1) // P  # 98
NQ = 4                 # emb quarter tables (int16 index range)
QROWS = N_SPOT // NQ   # 25000
SB = 4                 # blocks per superblock == blocks per cell
NSB = (NBLK + SB - 1) // SB  # 25
CALL_CAP = 8           # subtiles per dma_gather call
NQUEUES = 4            # SWDGE queues to round-robin
RING = 16384           # dynamic DMA scratch bytes (1024 descriptor ring)
PAD_SENTINEL = 3000.0  # outside [0, SB*P): pad edges match no one-hot column


def preprocess(emb, mask, call_cap=CALL_CAP):
    """Sort/shard/pad edges. Returns (in_maps, capsub, layout)."""
    qrows = QROWS
    emb = np.ascontiguousarray(np.asarray(emb, dtype=np.float32))
    emb16 = emb.astype(np.float16)
    mask = np.asarray(mask)
    src = mask[0].astype(np.int64, copy=False)
    dst = mask[1].astype(np.int64, copy=False)

    order = np.argsort(src, kind="stable")
    src_s = src[order].astype(np.int32)
    dst_s = dst[order].astype(np.int32)

    core_bounds = np.searchsorted(
        src_s, (SEG_PER_CORE * np.arange(NCORES + 1)).astype(np.int32)
    )

    ncell = NSB * NQ
    percore = []
    cnts = np.zeros((NCORES, ncell), np.int64)
    for k in range(NCORES):
        lo, hi = int(core_bounds[k]), int(core_bounds[k + 1])
        s = src_s[lo:hi] - SEG_PER_CORE * k
        d = dst_s[lo:hi]
        cell = (s >> 9) * NQ + d // qrows
        # sort by (cell, rel-block, d): rel-block grouping keeps subtile
        # spanning minimal; d-order within a block keeps the HBM gather
        # access pattern local (the gather is HBM-random-bound)
        o = np.lexsort((d, s >> 7, cell))
        s, d, cell = s[o], d[o], cell[o]
        cnts[k] = np.bincount(cell, minlength=ncell)
        percore.append((s, d, cell))

    capsub = (-(-cnts.max(axis=0) // P)).astype(np.int64)  # [ncell] subtiles
    nslots = int(capsub.sum())

    cell_slot0 = np.zeros(ncell, np.int64)
    cell_slot0[1:] = np.cumsum(capsub)[:-1]
    cell_base = cell_slot0 * P

    slot_mask = np.zeros(nslots, np.int64)  # rel-block bitmask, cross-core union

    # iota[p, (c*call_cap) + t] = c for c in [0, SB*P)
    iota512 = np.broadcast_to(
        np.repeat(np.arange(SB * P), call_cap).astype(np.float16)[None, :],
        (P, SB * P * call_cap),
    ).copy()

    # spread pad gather indices across the quarter: repeated fetches of a
    # single row serialize on an HBM bank (measured 2.7x slowdown)
    pad_spread = ((np.arange(nslots * P, dtype=np.int64) * 97) % qrows).astype(
        np.int16
    )

    in_maps = []
    for k in range(NCORES):
        s, d, cell = percore[k]
        cum = np.zeros(ncell, np.int64)
        cc = cnts[k]
        cum[1:] = np.cumsum(cc)[:-1]
        rank = np.arange(len(s), dtype=np.int64) - cum[cell]
        pos = cell_base[cell] + rank

        slot_g = cell_slot0[cell] + (rank >> 7)
        rb = (s >> 7) & (SB - 1)
        np.bitwise_or.at(slot_mask, slot_g, 1 << rb)

        srcloc = np.full(nslots * P, PAD_SENTINEL, np.float16)
        srcloc[pos] = (s & (SB * P - 1)).astype(np.float16)
        dloc = pad_spread.copy()
        dloc[pos] = (d % qrows).astype(np.int16)

        srcloc_t = np.ascontiguousarray(srcloc.reshape(nslots, P).T)
        # idx16 [j%16, slot*8 + j//16] = dloc of edge (slot, j), replicated
        # across the 8 partition groups for the Q7 ucode.
        idx_blk = np.ascontiguousarray(dloc.reshape(nslots * 8, 16).T)
        idx16 = np.tile(idx_blk, (8, 1))
        in_maps.append(
            {"emb": emb16, "srcloc": srcloc_t, "dstidx": idx16, "iota": iota512}
        )

    # layout: gather calls + per-call rb runs + per-block matmul lists
    calls = []           # (q, slot0, nsub)
    call_rb_runs = []    # per call: [(rb, t_lo, t_len)]
    blk_matmuls = [[] for _ in range(NBLK)]  # (call_idx, t, rb)
    sb_list = []         # (blocks, (call_lo, call_hi))
    for isb in range(NSB):
        blocks = list(range(isb * SB, min((isb + 1) * SB, NBLK)))
        call_lo = len(calls)
        for q in range(NQ):
            c = isb * NQ + q
            s0c = int(cell_slot0[c])
            cap = int(capsub[c])
            for i in range(0, cap, call_cap):
                nsub = min(call_cap, cap - i)
                s0 = s0c + i
                ci = len(calls)
                calls.append((q, s0, nsub))
                runs = []
                for rb in range(SB):
                    ts = [t for t in range(nsub)
                          if slot_mask[s0 + t] & (1 << rb)]
                    if not ts:
                        continue
                    # split into contiguous stretches (cross-core union of
                    # per-core contiguous ranges can, in principle, have gaps)
                    t_lo = ts[0]
                    prev = ts[0]
                    for t in ts[1:] + [None]:
                        if t is not None and t == prev + 1:
                            prev = t
                            continue
                        runs.append((rb, t_lo, prev - t_lo + 1))
                        if t is not None:
                            t_lo = prev = t
                    b = isb * SB + rb
                    for t in ts:
                        blk_matmuls[b].append((ci, t, rb))
                call_rb_runs.append(runs)
        sb_list.append((blocks, (call_lo, len(calls))))

    layout = {
        "nslots": nslots,
        "calls": calls,
        "call_rb_runs": call_rb_runs,
        "blk_matmuls": blk_matmuls,
        "sb_list": sb_list,
    }
    return in_maps, capsub, layout


def build_program(capsub, layout, repeats=1, call_cap=CALL_CAP, ring=RING,
                  gbufs=28, obufs=36, hw_loop=False):
    import concourse.bass as bass
    import concourse.tile as tile
    from concourse import bacc, mybir

    qrows = QROWS
    nslots = layout["nslots"]
    calls = layout["calls"]
    call_rb_runs = layout["call_rb_runs"]
    blk_matmuls = layout["blk_matmuls"]
    sb_list = layout["sb_list"]
    d = D

    nc = bacc.Bacc(
        "TRN2", target_bir_lowering=False, debug=False,
        num_swdge_queues=NQUEUES, dynamic_dma_scratch_size=ring,
    )
    emb_t = nc.dram_tensor("emb", [N_SPOT, d], mybir.dt.float16, kind="ExternalInput")
    srcloc_t = nc.dram_tensor(
        "srcloc", [P, nslots], mybir.dt.float16, kind="ExternalInput"
    )
    dstidx_t = nc.dram_tensor(
        "dstidx", [P, nslots * 8], mybir.dt.int16, kind="ExternalInput"
    )
    iota_t = nc.dram_tensor("iota", [P, SB * P * call_cap], mybir.dt.float16,
                            kind="ExternalInput")
    out_t = nc.dram_tensor(
        "out", [NBLK * P, d], mybir.dt.float32, kind="ExternalOutput"
    )

    with tile.TileContext(nc) as tc, ExitStack() as ctx:
        consts = ctx.enter_context(tc.tile_pool(name="consts", bufs=1))
        gpool = ctx.enter_context(tc.tile_pool(name="gather", bufs=gbufs))
        ohpool = ctx.enter_context(tc.tile_pool(name="onehot", bufs=obufs))
        spool = ctx.enter_context(tc.tile_pool(name="scratch", bufs=4))
        opool = ctx.enter_context(tc.tile_pool(name="outs", bufs=4))
        ppool = ctx.enter_context(tc.tile_pool(name="psum", bufs=8, space="PSUM"))

        srcloc_sb = consts.tile([P, nslots], mybir.dt.float16)
        nc.sync.dma_start(srcloc_sb[:], srcloc_t.ap())
        dstidx_sb = consts.tile([P, nslots * 8], mybir.dt.int16)
        nc.sync.dma_start(dstidx_sb[:], dstidx_t.ap())
        iota_sb = consts.tile([P, SB * P * call_cap], mybir.dt.float16)
        nc.sync.dma_start(iota_sb[:], iota_t.ap())

        out_ap = out_t.ap()
        emb_ap = emb_t.ap()

        def emit_body():
            callno = 0
            for blocks, (clo, chi) in sb_list:
                gtiles = {}
                ohtiles = {}
                for ci in range(clo, chi):
                    q, s0, nsub = calls[ci]
                    gt = gpool.tile([P, call_cap * d], mybir.dt.float16, tag="gt")
                    nc.gpsimd.dma_gather(
                        out_ap=gt[:, : nsub * d].rearrange(
                            "p (c e) -> p c e", e=d
                        ),
                        in_ap=emb_ap[q * qrows : (q + 1) * qrows, :],
                        idxs_ap=dstidx_sb[:, s0 * 8 : (s0 + nsub) * 8],
                        num_idxs=nsub * P,
                        num_idxs_reg=nsub * P,
                        elem_size=d,
                        single_packet=False,
                        queue_num=callno % NQUEUES,
                    )
                    gtiles[ci] = gt
                    callno += 1
                    iota_full = iota_sb[:, :]
                    srl0 = srcloc_sb[:, s0 : s0 + nsub]
                    for rb, t_lo, t_len in call_rb_runs[ci]:
                        oh = ohpool.tile([P, call_cap * P], mybir.dt.float16,
                                         tag="oh")
                        full = oh[:, :]
                        oh3 = bass.AP(
                            full.tensor, full.offset + t_lo,
                            [full.ap[0], [call_cap, P], [1, t_len]],
                        )
                        iota_b = bass.AP(
                            iota_full.tensor,
                            iota_full.offset + rb * P * call_cap + t_lo,
                            [iota_full.ap[0], [call_cap, P], [1, t_len]],
                        )
                        srl_b = bass.AP(
                            srl0.tensor, srl0.offset + t_lo,
                            [srl0.ap[0], [0, P], [1, t_len]],
                        )
                        nc.vector.tensor_tensor(
                            out=oh3, in0=iota_b, in1=srl_b,
                            op=mybir.AluOpType.is_equal,
                        )
                        ohtiles[(ci, rb)] = oh
                for b in blocks:
                    mms = blk_matmuls[b]
                    if not mms:
                        ot = opool.tile([P, d], mybir.dt.float32)
                        nc.vector.memset(ot[:], 0.0)
                        nc.sync.dma_start(out_ap[b * P : (b + 1) * P, :], ot[:])
                        continue
                    ps = ppool.tile([P, d], mybir.dt.float32, space="PSUM")
                    for i, (ci, t, rb) in enumerate(mms):
                        ohfull = ohtiles[(ci, rb)][:, :]
                        lhsT = bass.AP(
                            ohfull.tensor, ohfull.offset + t,
                            [ohfull.ap[0], [call_cap, P]],
                        )
                        nc.tensor.matmul(
                            ps[:],
                            lhsT=lhsT,
                            rhs=gtiles[ci][:, t * d : (t + 1) * d],
                            start=(i == 0),
                            stop=(i == len(mms) - 1),
                        )
                    sq = spool.tile([P, d], mybir.dt.float32)
                    ss = spool.tile([P, 1], mybir.dt.float32)
                    nc.scalar.activation(
                        sq[:], ps[:], mybir.ActivationFunctionType.Square,
                        accum_out=ss[:],
                    )
                    nrm = spool.tile([P, 1], mybir.dt.float32)
                    nc.scalar.activation(
                        nrm[:], ss[:], mybir.ActivationFunctionType.Sqrt
                    )
                    nc.vector.tensor_scalar(
                        out=nrm[:], in0=nrm[:], scalar1=1e-12, scalar2=None,
                        op0=mybir.AluOpType.max,
                    )
                    nc.vector.reciprocal(nrm[:], nrm[:])
                    ot = opool.tile([P, d], mybir.dt.float32)
                    nc.scalar.activation(
                        ot[:], ps[:], mybir.ActivationFunctionType.Copy,
                        scale=nrm[:],
                    )
                    nc.sync.dma_start(out_ap[b * P : (b + 1) * P, :], ot[:])

        if hw_loop and repeats > 1:
            assert repeats % hw_loop == 0
            with tc.For_i(0, repeats // hw_loop) as _i:
                for _u in range(hw_loop):
                    emit_body()
        else:
            for _rep in range(repeats):
                emit_body()

    nc.compile()
    return nc


_PROGRAM_CACHE = {}


def _get_program(capsub, layout, **kw):
    key = (capsub.tobytes(), tuple(sorted(kw.items())))
    if key not in _PROGRAM_CACHE:
        _PROGRAM_CACHE[key] = build_program(capsub, layout, **kw)
    return _PROGRAM_CACHE[key]


def kernel(**inputs):
    emb = inputs["emb"]
    mask = inputs["mask"]
    in_maps, capsub, layout = preprocess(emb, mask)
    nc = _get_program(capsub, layout)

    from concourse.bass_utils import run_bass_kernel_spmd

    res = run_bass_kernel_spmd(nc, in_maps, core_ids=list(range(NCORES)))
    out = np.empty((N_SPOT, D), np.float32)
    for k in range(NCORES):
        out[k * SEG_PER_CORE : (k + 1) * SEG_PER_CORE] = res.results[k]["out"][
            :SEG_PER_CORE
        ]
    return out
